# revision 16
# baseline (speedup 1.0000x reference)
"""Pointer Network Bass kernel for 8 Trainium2 NeuronCores.

Sharding: pure data-parallel over batch (B=64 -> 8 per core), weights
replicated. Recurrent state lives transposed ([128, 4x8] = h.T) so gate
math is partition-rich; matmuls are stationary-weight f32r tiles; the
attention dot splits vt into tf32 hi/lo to halve rounding error; the
log_softmax over pointer scores is deferred to a batched final pass.
"""
import sys
import os
import numpy as np

sys.path.insert(0, "/opt/trn_rl_repo")

B, S, E, H = 64, 256, 256, 512
G = 4 * H  # 2048
NCORES = 8
BC = B // NCORES  # 8
LOG_EPS = float(np.log(np.float32(1e-45)))
NEG_BIG = -1e38

_CACHE = {}


def _round_tf32(x):
    a = np.ascontiguousarray(x, np.float32).view(np.uint32)
    rnd = ((a >> np.uint32(13)) & np.uint32(1)) + np.uint32(0x0FFF)
    return ((a + rnd) & np.uint32(0xFFFFE000)).view(np.float32)


# consts tile column offsets
COFS = {}
_o = 0
for _n, _w in [("be", 128), ("bd", 128), ("lenb", 32), ("mka", 256), ("mkb", 256),
               ("vlog", 256), ("indc", 16), ("s2c", 16), ("lenrow", 8), ("ident", 128),
               ("iotas", 256), ("lenp", 1)]:
    COFS[_n] = _o
    _o += _w
CCOLS = _o


def _build():
    import concourse.bacc as bacc
    import concourse.mybir as mybir
    import concourse.tile as tile
    from concourse.bass import IndirectOffsetOnAxis

    dt = mybir.dt
    AF = mybir.ActivationFunctionType
    OP = mybir.AluOpType

    nc = bacc.Bacc(trn_type="TRN2")

    wenc_d = nc.declare_dram_parameter("wenc", [128, 6 * G], dt.float32, isOutput=False)
    wdec_d = nc.declare_dram_parameter("wdec", [128, 8 * G], dt.float32, isOutput=False)
    wsq_d = nc.declare_dram_parameter("wsq", [128, 2 * 2048 + 8], dt.float32, isOutput=False)
    consts_d = nc.declare_dram_parameter("consts", [128, CCOLS], dt.float32, isOutput=False)
    embt_d = nc.declare_dram_parameter("embT", [128, 8 * E], dt.float32, isOutput=False)
    ohsrc_d = nc.declare_dram_parameter("ohsrc", [128, 8 * S * BC], dt.float32, isOutput=False)

    scores_d = nc.declare_dram_parameter("scores", [S, BC, S], dt.float32, isOutput=True)
    idx_d = nc.declare_dram_parameter("idx", [1, S * BC], dt.uint32, isOutput=True)

    raws_d = nc.dram_tensor("raws", [S, BC, S], dt.float32)

    with tile.TileContext(nc) as tc:
        with tc.tile_pool(name="const", bufs=1) as cpool, \
             tc.tile_pool(name="state", bufs=1) as spool:
            ct = cpool.tile([128, CCOLS], dt.float32)
            nc.sync.dma_start(out=ct[:], in_=consts_d[:])
            def cs(name, w):
                return ct[:, COFS[name]:COFS[name] + w]

            be_t = cs("be", 128)
            bd_t = cs("bd", 128)
            lenb = cs("lenb", 32)
            mka = cs("mka", 256)
            mkb = cs("mkb", 256)
            vlog = cs("vlog", 256)
            indc = cs("indc", 16)
            s2c = cs("s2c", 16)
            lenrow = ct[0:1, COFS["lenrow"]:COFS["lenrow"] + 8]
            ident = cs("ident", 128)
            iotas = ct[0:8, COFS["iotas"]:COFS["iotas"] + 256]
            lenp = ct[0:8, COFS["lenp"]:COFS["lenp"] + 1]

            w2t = cpool.tile([128, 2048], dt.float32r)
            vtt = cpool.tile([128, 8], dt.float32)

            uT = cpool.tile([128, 4 * 2048], dt.float32)

            hT = spool.tile([128, 32], dt.float32)
            cT = spool.tile([128, 32], dt.float32)
            xhr = spool.tile([128, 64], dt.float32r)
            henc_r = spool.tile([128, 32], dt.float32r)
            nc.gpsimd.memset(hT[:], 0.0)
            nc.gpsimd.memset(cT[:], 0.0)
            nc.vector.tensor_scalar(xhr[:], ct[:, 0:64], 0.0, None, OP.mult)
            nc.vector.tensor_scalar(henc_r[:], ct[:, 0:32], 0.0, None, OP.mult)

            rowsp_cm = tc.tile_pool(name="rowsp", bufs=1)
            rowsp = rowsp_cm.__enter__()
            encrows_sb = rowsp.tile([128, BC * 2 * 512], dt.float32)
            with tc.tile_pool(name="eo", bufs=1) as eop:
                encw_cm = tc.tile_pool(name="encw", bufs=1)
                encw = encw_cm.__enter__()
                enc_outT = eop.tile([128, S * 32], dt.float32)  # (s, k, b)
                w1t = eop.tile([128, 2048], dt.float32r)
                with tc.tile_pool(name="wsqp", bufs=1) as wsqp:
                    wsq32 = wsqp.tile([128, 2 * 2048 + 8], dt.float32)
                    nc.sync.dma_start(out=wsq32[:], in_=wsq_d[:])
                    nc.vector.tensor_copy(w2t[:], wsq32[:, 0:2048])
                    nc.vector.tensor_copy(w1t[:], wsq32[:, 2048:4096])
                    nc.vector.tensor_copy(vtt[:], wsq32[:, 4096:4104])
                enc_lhsT = encw.tile([128, 6 * G], dt.float32r)
                xT = encw.tile([128, 2 * S * BC], dt.float32r)  # (ec; s,b)

                with tc.tile_pool(name="wtmp", bufs=2) as wtmp:
                    for kc in range(6):
                        we32 = wtmp.tile([128, G], dt.float32, tag="we32")
                        nc.sync.dma_start(out=we32[:], in_=wenc_d[:, kc * G:(kc + 1) * G])
                        nc.vector.tensor_copy(enc_lhsT[:, kc * G:(kc + 1) * G], we32[:])

                # ---- embedding via one-hot matmul (exact fp32 gather)
                with tc.tile_pool(name="embp", bufs=1) as embp, \
                     tc.tile_pool(name="ohp", bufs=1) as ohp, \
                     tc.tile_pool(name="pse", bufs=1, space="PSUM") as pse:
                    embt = embp.tile([128, 8 * E], dt.float32)
                    nc.sync.dma_start(out=embt[:], in_=embt_d[:])
                    pe0 = pse.tile([128, S * BC], dt.float32, tag="pe0")
                    pe1 = pse.tile([128, S * BC], dt.float32, tag="pe1")
                    pes = [pe0, pe1]
                    for kc in range(8):
                        ohc = ohp.tile([128, S * BC], dt.float32, tag="ohc")
                        nc.sync.dma_start(out=ohc[:],
                                          in_=ohsrc_d[:, kc * S * BC:(kc + 1) * S * BC])
                        for m in range(2):
                            for ns in range(4):
                                nc.tensor.matmul(
                                    pes[m][:, 512 * ns:512 * (ns + 1)],
                                    embt[:, kc * E + 128 * m: kc * E + 128 * (m + 1)],
                                    ohc[:, 512 * ns:512 * (ns + 1)],
                                    start=(kc == 0), stop=(kc == 7))
                    for m in range(2):
                        nc.vector.tensor_copy(
                            xT[:, m * S * BC:(m + 1) * S * BC], pes[m][:])

                # ---- encoder ----
                with tc.tile_pool(name="psg", bufs=2, space="PSUM") as psg, \
                     tc.tile_pool(name="esb", bufs=3) as esb:
                    for t in range(S):
                        pg = psg.tile([128, 128], dt.float32, tag="pg")
                        for kc in [2, 3, 4, 5, 0, 1]:
                            if kc < 2:
                                rhs = xT[:, kc * (S * BC) + t * BC:
                                         kc * (S * BC) + (t + 1) * BC]
                            else:
                                rhs = henc_r[:, 8 * (kc - 2): 8 * (kc - 2) + 8]
                            for m in range(16):
                                nc.tensor.matmul(
                                    pg[:, 8 * m:8 * m + 8],
                                    enc_lhsT[:, kc * G + 128 * m: kc * G + 128 * (m + 1)],
                                    rhs, start=(kc == 2), stop=(kc == 1))
                        pre = esb.tile([128, 128], dt.float32, tag="pre")
                        nc.vector.tensor_tensor(out=pre[:], in0=pg[:], in1=be_t, op=OP.add)
                        sg = esb.tile([128, 96], dt.float32, tag="sg")
                        tg = esb.tile([128, 32], dt.float32, tag="tg")
                        nc.scalar.activation(sg[:], pre[:, 0:96], AF.Sigmoid)
                        nc.scalar.activation(tg[:], pre[:, 96:128], AF.Tanh)
                        t1 = esb.tile([128, 32], dt.float32, tag="t1")
                        c2 = esb.tile([128, 32], dt.float32, tag="c2")
                        nc.vector.tensor_tensor(out=t1[:], in0=sg[:, 0:32], in1=tg[:], op=OP.mult)
                        nc.vector.tensor_tensor(out=c2[:], in0=sg[:, 32:64], in1=cT[:], op=OP.mult)
                        nc.vector.tensor_tensor(out=c2[:], in0=c2[:], in1=t1[:], op=OP.add)
                        tcc = esb.tile([128, 32], dt.float32, tag="tcc")
                        nc.scalar.activation(tcc[:], c2[:], AF.Tanh)
                        h2 = esb.tile([128, 32], dt.float32, tag="h2")
                        nc.vector.tensor_tensor(out=h2[:], in0=sg[:, 64:96], in1=tcc[:], op=OP.mult)
                        msk = esb.tile([128, 32], dt.uint8, tag="msk")
                        mskf32 = esb.tile([128, 32], dt.float32, tag="mskf32")
                        nc.vector.tensor_scalar(msk[:], lenb, float(t), None, OP.is_gt)
                        nc.vector.tensor_scalar(mskf32[:], lenb, float(t), None, OP.is_gt)
                        nc.vector.select(cT[:], msk[:], c2[:], cT[:])
                        nc.vector.select(hT[:], msk[:], h2[:], hT[:])
                        et = enc_outT[:, 32 * t:32 * t + 32]
                        nc.vector.tensor_tensor(out=et, in0=mskf32[:], in1=hT[:], op=OP.mult)
                        nc.vector.tensor_copy(henc_r[:], hT[:])

                encw_cm.__exit__(None, None, None)
                # ---- reformat enc_out to SBUF row form (xnext one-hot source)
                with tc.tile_pool(name="rfps", bufs=2, space="PSUM") as rfps:
                    eview = enc_outT[:].rearrange("p (s k b) -> p k b s", s=S, k=4, b=BC)
                    for b in range(BC):
                        for sh in range(2):
                            for k in range(4):
                                pt = rfps.tile([128, 128], dt.float32, tag="pt")
                                nc.tensor.transpose(
                                    pt[:], eview[:, k, b, 128 * sh:128 * (sh + 1)], ident)
                                nc.vector.tensor_copy(
                                    encrows_sb[:, b * 1024 + sh * 512 + 128 * k:
                                               b * 1024 + sh * 512 + 128 * (k + 1)], pt[:])

                # ---- U = W1 @ enc_out.T ----
                with tc.tile_pool(name="uc", bufs=1) as ucp, \
                     tc.tile_pool(name="psu", bufs=2, space="PSUM") as psu:
                    encr = ucp.tile([128, S * 32], dt.float32r)
                    nc.vector.tensor_copy(encr[:], enc_outT[:])
                    erv = encr[:].rearrange("p (s k b) -> p k b s", s=S, k=4, b=BC)
                    for mc in range(4):
                        for nb in range(4):
                            pu = psu.tile([128, 512], dt.float32, tag="pu")
                            for kc in range(4):
                                nc.tensor.matmul(
                                    pu[:],
                                    w1t[:, kc * 512 + 128 * mc: kc * 512 + 128 * (mc + 1)],
                                    erv[:, kc, 2 * nb:2 * nb + 2, :],
                                    start=(kc == 0), stop=(kc == 3))
                            nc.vector.tensor_copy(
                                uT[:, mc * 2048 + nb * 512: mc * 2048 + (nb + 1) * 512],
                                pu[:])

            # ---- decoder ----
            with tc.tile_pool(name="decw", bufs=1) as decw:
                dec_lhsT = decw.tile([128, 8 * G], dt.float32r)
                with tc.tile_pool(name="wtmp2", bufs=2) as wtmp2:
                    for kc in range(8):
                        wd32 = wtmp2.tile([128, G], dt.float32, tag="wd32")
                        nc.sync.dma_start(out=wd32[:], in_=wdec_d[:, kc * G:(kc + 1) * G])
                        nc.vector.tensor_copy(dec_lhsT[:, kc * G:(kc + 1) * G], wd32[:])

                with tc.tile_pool(name="psd", bufs=2, space="PSUM") as psd, \
                     tc.tile_pool(name="psq", bufs=1, space="PSUM") as psq, \
                     tc.tile_pool(name="ptxp", bufs=1, space="PSUM") as ptxp, \
                     tc.tile_pool(name="ptrp", bufs=1, space="PSUM") as ptrp, \
                     tc.tile_pool(name="pdt", bufs=2, space="PSUM") as pdtp, \
                     tc.tile_pool(name="dsb", bufs=2) as dsb:
                    nc.vector.tensor_copy(xhr[:, 32:64], hT[:])
                    pg_prev = None
                    for i in range(S):
                        if pg_prev is None:
                            pg = psd.tile([128, 128], dt.float32, tag="pg")
                            for kc in [4, 5, 6, 7]:
                                rhs = xhr[:, 8 * kc: 8 * kc + 8]
                                for m in range(16):
                                    nc.tensor.matmul(
                                        pg[:, 8 * m:8 * m + 8],
                                        dec_lhsT[:, kc * G + 128 * m: kc * G + 128 * (m + 1)],
                                        rhs, start=(kc == 4), stop=False)
                        else:
                            pg = pg_prev
                        for kc in [0, 1, 2, 3]:
                            rhs = xhr[:, 8 * kc: 8 * kc + 8]
                            for m in range(16):
                                nc.tensor.matmul(
                                    pg[:, 8 * m:8 * m + 8],
                                    dec_lhsT[:, kc * G + 128 * m: kc * G + 128 * (m + 1)],
                                    rhs, start=False, stop=(kc == 3))
                        pre = dsb.tile([128, 128], dt.float32, tag="pre")
                        nc.vector.tensor_tensor(out=pre[:], in0=pg[:], in1=bd_t, op=OP.add)
                        sg = dsb.tile([128, 96], dt.float32, tag="sg")
                        tg = dsb.tile([128, 32], dt.float32, tag="tg")
                        nc.scalar.activation(sg[:], pre[:, 0:96], AF.Sigmoid)
                        nc.scalar.activation(tg[:], pre[:, 96:128], AF.Tanh)
                        t1 = dsb.tile([128, 32], dt.float32, tag="t1")
                        nc.vector.tensor_tensor(out=t1[:], in0=sg[:, 0:32], in1=tg[:], op=OP.mult)
                        nc.vector.tensor_tensor(out=cT[:], in0=sg[:, 32:64], in1=cT[:], op=OP.mult)
                        nc.vector.tensor_tensor(out=cT[:], in0=cT[:], in1=t1[:], op=OP.add)
                        tcc = dsb.tile([128, 32], dt.float32, tag="tcc")
                        nc.scalar.activation(tcc[:], cT[:], AF.Tanh)
                        nc.vector.tensor_tensor(out=hT[:], in0=sg[:, 64:96], in1=tcc[:], op=OP.mult)
                        nc.vector.tensor_copy(xhr[:, 32:64], hT[:])

                        pq = psq.tile([128, 32], dt.float32, tag="pq")
                        for kc in range(4):
                            rhs = xhr[:, 32 + 8 * kc: 32 + 8 * kc + 8]
                            for m in range(4):
                                nc.tensor.matmul(
                                    pq[:, 8 * m:8 * m + 8],
                                    w2t[:, kc * 512 + 128 * m: kc * 512 + 128 * (m + 1)],
                                    rhs, start=(kc == 0), stop=(kc == 3))
                        qT = dsb.tile([128, 32], dt.float32, tag="qT")
                        nc.vector.tensor_copy(qT[:], pq[:])

                        pd = pdtp.tile([128, 512], dt.float32, tag="pd")
                        for c in range(4):
                            att = dsb.tile([128, 2048], dt.float32, tag="att")
                            for b in range(BC):
                                nc.vector.tensor_scalar(
                                    att[:, 256 * b:256 * (b + 1)],
                                    uT[:, c * 2048 + 256 * b: c * 2048 + 256 * (b + 1)],
                                    qT[:, 8 * c + b: 8 * c + b + 1], None, OP.add)
                            th = dsb.tile([128, 2048], dt.float32, tag="th")
                            nc.scalar.activation(th[:], att[:], AF.Tanh)
                            for ti in range(2):
                                for j in range(4):
                                    b = 4 * ti + j
                                    nc.tensor.matmul(
                                        pd[32 * j:32 * j + 1, 256 * ti:256 * (ti + 1)],
                                        vtt[:, c: c + 1],
                                        th[:, 256 * b:256 * (b + 1)],
                                        start=(c == 0),
                                        stop=(c == 3),
                                        tile_position=(0, 32 * j))

                        if i + 1 < S:
                            pg_prev = psd.tile([128, 128], dt.float32, tag="pg")
                            for kc in [4, 5, 6, 7]:
                                rhs = xhr[:, 8 * kc: 8 * kc + 8]
                                for m in range(16):
                                    nc.tensor.matmul(
                                        pg_prev[:, 8 * m:8 * m + 8],
                                        dec_lhsT[:, kc * G + 128 * m: kc * G + 128 * (m + 1)],
                                        rhs, start=(kc == 4), stop=False)

                        raw0 = dsb.tile([128, 256], dt.float32, tag="raw0")
                        raw1 = dsb.tile([128, 256], dt.float32, tag="raw1")
                        nc.vector.tensor_copy(raw0[:], pd[:, 0:256])
                        nc.vector.tensor_copy(raw1[:], pd[:, 256:512])
                        nc.sync.dma_start(out=raws_d[i, 0:4, :], in_=raw0[0:128:32, :])
                        nc.sync.dma_start(out=raws_d[i, 4:8, :], in_=raw1[0:128:32, :])

                        sc0 = dsb.tile([128, 256], dt.float32, tag="sc0")
                        sc1 = dsb.tile([128, 256], dt.float32, tag="sc1")
                        nc.vector.tensor_tensor(out=sc0[:], in0=pd[:, 0:256], in1=mka, op=OP.add)
                        nc.vector.tensor_tensor(out=sc1[:], in0=pd[:, 256:512], in1=mkb, op=OP.add)
                        mx0 = dsb.tile([128, 8], dt.float32, tag="mx0")
                        mi0 = dsb.tile([128, 8], dt.uint32, tag="mi0")
                        mx1 = dsb.tile([128, 8], dt.float32, tag="mx1")
                        mi1 = dsb.tile([128, 8], dt.uint32, tag="mi1")
                        nc.vector.max_with_indices(mx0[:], mi0[:], sc0[:])
                        nc.vector.max_with_indices(mx1[:], mi1[:], sc1[:])
                        idxP = dsb.tile([8, 1], dt.uint32, tag="idxP")
                        nc.sync.dma_start(out=idxP[0:4, 0:1], in_=mi0[0:128:32, 0:1])
                        nc.sync.dma_start(out=idxP[4:8, 0:1], in_=mi1[0:128:32, 0:1])
                        indP = dsb.tile([8, 1], dt.float32, tag="indP")
                        nc.vector.tensor_scalar(indP[:], lenp, float(i), None, OP.is_gt)
                        idxPf = dsb.tile([8, 1], dt.float32, tag="idxPf")
                        nc.vector.tensor_copy(idxPf[:], idxP[:])
                        idxPm = dsb.tile([8, 1], dt.float32, tag="idxPm")
                        nc.vector.tensor_tensor(out=idxPm[:], in0=idxPf[:], in1=indP[:],
                                                op=OP.mult)
                        idxu = dsb.tile([8, 1], dt.uint32, tag="idxu")
                        nc.vector.tensor_copy(idxu[:], idxPm[:])
                        nc.sync.dma_start(out=idx_d[0:1, BC * i: BC * i + BC],
                                          in_=idxu[:])
                        ohT = dsb.tile([8, 256], dt.float32, tag="ohT")
                        nc.vector.tensor_scalar(ohT[:], iotas, idxPm[:], None, OP.is_equal)
                        ptr = ptrp.tile([128, 16], dt.float32, tag="ptr")
                        for sh in range(2):
                            nc.tensor.transpose(ptr[:, 8 * sh:8 * sh + 8],
                                                ohT[:, 128 * sh:128 * (sh + 1)],
                                                ident[0:8, 0:8])
                        ohsb = dsb.tile([128, 16], dt.float32, tag="ohsb")
                        nc.vector.tensor_copy(ohsb[:], ptr[:])
                        ptx = ptxp.tile([128, 32], dt.float32, tag="ptx")
                        for b in range(BC):
                            for m in range(4):
                                for sh in range(2):
                                    nc.tensor.matmul(
                                        ptx[:, 8 * m + b: 8 * m + b + 1],
                                        encrows_sb[:, b * 1024 + sh * 512 + 128 * m:
                                                   b * 1024 + sh * 512 + 128 * (m + 1)],
                                        ohsb[:, sh * 8 + b: sh * 8 + b + 1],
                                        start=(sh == 0), stop=(sh == 1))
                        nc.vector.tensor_copy(xhr[:, 0:32], ptx[:])

            rowsp_cm.__exit__(None, None, None)
            # ---- final pass: masked log_softmax ----
            with tc.tile_pool(name="fin", bufs=3) as fsb:
                for ch in range(16):
                    raw = fsb.tile([128, 256], dt.float32, tag="raw")
                    src = raws_d[:].rearrange("(ch q) b s -> ch (q b) s", ch=16, q=16)
                    nc.sync.dma_start(out=raw[:], in_=src[ch])
                    t1f = fsb.tile([128, 256], dt.float32, tag="ft1")
                    nc.vector.tensor_scalar(t1f[:], vlog, indc[:, ch:ch + 1], None, OP.mult)
                    nc.vector.tensor_scalar(t1f[:], t1f[:], s2c[:, ch:ch + 1], None, OP.add)
                    mskf = fsb.tile([128, 256], dt.float32, tag="fmsk")
                    nc.vector.tensor_tensor(out=mskf[:], in0=raw[:], in1=t1f[:], op=OP.add)
                    mx = fsb.tile([128, 1], dt.float32, tag="fmx")
                    nc.vector.tensor_reduce(out=mx[:], in_=mskf[:],
                                            axis=mybir.AxisListType.X, op=OP.max)
                    nmx = fsb.tile([128, 1], dt.float32, tag="fnmx")
                    nc.vector.tensor_scalar(nmx[:], mx[:], -1.0, None, OP.mult)
                    ex = fsb.tile([128, 256], dt.float32, tag="fex")
                    se = fsb.tile([128, 1], dt.float32, tag="fse")
                    nc.scalar.activation(ex[:], mskf[:], AF.Exp, bias=nmx[:], scale=1.0,
                                         accum_out=se[:])
                    ls = fsb.tile([128, 1], dt.float32, tag="fls")
                    nc.scalar.activation(ls[:], se[:], AF.Ln)
                    tot = fsb.tile([128, 1], dt.float32, tag="ftot")
                    nc.vector.tensor_tensor(out=tot[:], in0=mx[:], in1=ls[:], op=OP.add)
                    outt = fsb.tile([128, 256], dt.float32, tag="fout")
                    nc.vector.tensor_scalar(outt[:], mskf[:], tot[:], None, OP.subtract)
                    dstv = scores_d[:].rearrange("(ch q) b s -> ch (q b) s", ch=16, q=16)
                    nc.sync.dma_start(out=dstv[ch], in_=outt[:])

    nc.finalize()
    return nc


def _prep(inputs):
    src = np.asarray(inputs["src"], np.int32)
    src_len = np.asarray(inputs["src_len"], np.int32)
    emb = np.asarray(inputs["emb"], np.float32)
    W_ih_e = np.asarray(inputs["W_ih_e"], np.float32)
    W_hh_e = np.asarray(inputs["W_hh_e"], np.float32)
    b_e = np.asarray(inputs["b_e"], np.float32)
    W_ih_d = np.asarray(inputs["W_ih_d"], np.float32)
    W_hh_d = np.asarray(inputs["W_hh_d"], np.float32)
    b_d = np.asarray(inputs["b_d"], np.float32)
    W1 = np.asarray(inputs["W1"], np.float32)
    W2 = np.asarray(inputs["W2"], np.float32)
    vt = np.asarray(inputs["vt"], np.float32)

    perm = np.concatenate([np.arange(0, 512), np.arange(512, 1024),
                           np.arange(1536, 2048), np.arange(1024, 1536)])

    def lhsT_tiles(Wf, nk):
        t = Wf.T.reshape(nk, 128, G)
        return np.ascontiguousarray(t.transpose(1, 0, 2).reshape(128, nk * G))

    wenc = lhsT_tiles(np.concatenate([W_ih_e[perm], W_hh_e[perm]], axis=1), 6)
    wdec = lhsT_tiles(np.concatenate([W_ih_d[perm], W_hh_d[perm]], axis=1), 8)

    emb_pad = np.zeros((1024, E), np.float32)
    emb_pad[0:1001] = emb
    embT = np.zeros((128, 8 * E), np.float32)
    for kc in range(8):
        embT[:, kc * E:(kc + 1) * E] = emb_pad[128 * kc:128 * (kc + 1), :]

    def sq_tiles(Wm):
        t = Wm.T.reshape(4, 128, 512)
        return np.ascontiguousarray(t.transpose(1, 0, 2).reshape(128, 4 * 512))

    vt_pack = np.zeros((128, 8), np.float32)
    for kc in range(4):
        vt_pack[:, kc] = vt[128 * kc:128 * (kc + 1)]
    wsq = np.concatenate([sq_tiles(W2), sq_tiles(W1), vt_pack], axis=1)

    bep = b_e[perm]
    bdp = b_d[perm]
    be_tile = np.zeros((128, 128), np.float32)
    bd_tile = np.zeros((128, 128), np.float32)
    for m in range(16):
        be_tile[:, 8 * m:8 * m + 8] = bep[128 * m:128 * (m + 1)][:, None]
        bd_tile[:, 8 * m:8 * m + 8] = bdp[128 * m:128 * (m + 1)][:, None]

    in_maps = []
    auxs = []
    for core in range(NCORES):
        sl = src_len[core * BC:(core + 1) * BC]
        sv = src[core * BC:(core + 1) * BC]

        lenb = np.zeros((128, 32), np.float32)
        for k in range(4):
            lenb[:, 8 * k:8 * k + 8] = sl[None, :].astype(np.float32)

        valid = (np.arange(S)[None, :] < sl[:, None])  # [BC, S]
        mka = np.zeros((128, 256), np.float32)
        mkb = np.zeros((128, 256), np.float32)
        for j in range(4):
            mka[32 * j] = np.where(valid[j], 0.0, NEG_BIG)
            mkb[32 * j] = np.where(valid[4 + j], 0.0, NEG_BIG)

        vlogv = np.zeros((128, 256), np.float32)
        for p in range(128):
            vlogv[p] = np.where(valid[p % 8], 0.0, LOG_EPS)
        indcv = np.zeros((128, 16), np.float32)
        s2cv = np.zeros((128, 16), np.float32)
        for ch in range(16):
            for p in range(128):
                row = ch * 128 + p
                ind = 1.0 if (row // 8) < sl[row % 8] else 0.0
                indcv[p, ch] = ind
                s2cv[p, ch] = LOG_EPS * (1.0 - ind)

        lenrowv = np.zeros((128, 8), np.float32)
        lenrowv[0] = sl.astype(np.float32)
        identv = np.eye(128, dtype=np.float32)
        iotasv = np.zeros((128, 256), np.float32)
        iotasv[0:8] = np.arange(256, dtype=np.float32)[None, :]
        lenpv = np.zeros((128, 1), np.float32)
        lenpv[0:8, 0] = sl.astype(np.float32)
        consts = np.concatenate([be_tile, bd_tile, lenb, mka, mkb, vlogv,
                                 indcv, s2cv, lenrowv, identv, iotasv, lenpv], axis=1)

        # one-hot of src tokens: [128, 8*2048], tile kc col sb -> (row==128*kc+p)
        sbord = sv.T.reshape(-1)  # (s,b) -> token
        ohsrc = np.zeros((128, 8 * S * BC), np.float32)
        rows = sbord.astype(np.int64)
        for kc in range(8):
            inblk = (rows >= 128 * kc) & (rows < 128 * (kc + 1))
            cols = np.nonzero(inblk)[0]
            ohsrc[rows[cols] - 128 * kc, kc * S * BC + cols] = 1.0

        in_maps.append({"wenc": wenc, "wdec": wdec, "wsq": wsq.astype(np.float32),
                        "consts": consts.astype(np.float32),
                        "embT": embT, "ohsrc": ohsrc})
        auxs.append({"valid": valid})
    return in_maps, auxs


def kernel(**inputs):
    import concourse.bass_utils as bass_utils

    if "nc" not in _CACHE:
        _CACHE["nc"] = _build()
    nc = _CACHE["nc"]

    in_maps, auxs = _prep(inputs)
    res = bass_utils.run_bass_kernel_spmd(nc, in_maps, list(range(NCORES)))

    scores = np.zeros((B, S, S), np.float32)
    idxs = np.zeros((B, S), np.int32)
    valid_out = np.zeros((B, S), np.int32)
    for core in range(NCORES):
        out = res.results[core]
        scores[core * BC:(core + 1) * BC] = out["scores"].transpose(1, 0, 2)
        idxs[core * BC:(core + 1) * BC] = out["idx"].reshape(S, BC).T.astype(np.int32)
        valid_out[core * BC:(core + 1) * BC] = auxs[core]["valid"].astype(np.int32)
    return scores, idxs, valid_out


if __name__ == "__main__":
    data = dict(np.load(os.path.join(os.path.dirname(__file__), "inputs.npz")))
    out = kernel(**data)
    print([o.shape for o in out])


# revision 17
# speedup vs baseline: 1.0629x; 1.0629x over previous
"""Pointer Network Bass kernel for 8 Trainium2 NeuronCores.

Sharding: pure data-parallel over batch (B=64 -> 8 per core), weights
replicated. Recurrent state lives transposed ([128, 4x8] = h.T) so gate
math is partition-rich; matmuls are stationary-weight f32r tiles; the
attention dot splits vt into tf32 hi/lo to halve rounding error; the
log_softmax over pointer scores is deferred to a batched final pass.
"""
import sys
import os
import numpy as np

sys.path.insert(0, "/opt/trn_rl_repo")

B, S, E, H = 64, 256, 256, 512
G = 4 * H  # 2048
NCORES = 8
BC = B // NCORES  # 8
LOG_EPS = float(np.log(np.float32(1e-45)))
NEG_BIG = -1e38

_CACHE = {}


def _round_tf32(x):
    a = np.ascontiguousarray(x, np.float32).view(np.uint32)
    rnd = ((a >> np.uint32(13)) & np.uint32(1)) + np.uint32(0x0FFF)
    return ((a + rnd) & np.uint32(0xFFFFE000)).view(np.float32)


# consts tile column offsets
COFS = {}
_o = 0
for _n, _w in [("be", 128), ("bd", 128), ("lenb", 32), ("mka", 256), ("mkb", 256),
               ("vlog", 256), ("indc", 16), ("s2c", 16), ("lenrow", 8), ("ident", 128),
               ("iotas", 256), ("lenp", 1)]:
    COFS[_n] = _o
    _o += _w
CCOLS = _o


def _build():
    import concourse.bacc as bacc
    import concourse.mybir as mybir
    import concourse.tile as tile
    from concourse.bass import IndirectOffsetOnAxis

    dt = mybir.dt
    AF = mybir.ActivationFunctionType
    OP = mybir.AluOpType

    nc = bacc.Bacc(trn_type="TRN2")

    wenc_d = nc.declare_dram_parameter("wenc", [128, 6 * G], dt.float32, isOutput=False)
    wdec_d = nc.declare_dram_parameter("wdec", [128, 8 * G], dt.float32, isOutput=False)
    wsq_d = nc.declare_dram_parameter("wsq", [128, 2 * 2048 + 8], dt.float32, isOutput=False)
    consts_d = nc.declare_dram_parameter("consts", [128, CCOLS], dt.float32, isOutput=False)
    embt_d = nc.declare_dram_parameter("embT", [128, 8 * E], dt.float32, isOutput=False)
    ohsrc_d = nc.declare_dram_parameter("ohsrc", [128, 8 * S * BC], dt.float32, isOutput=False)

    scores_d = nc.declare_dram_parameter("scores", [S, BC, S], dt.float32, isOutput=True)
    idx_d = nc.declare_dram_parameter("idx", [1, S * BC], dt.uint32, isOutput=True)

    raws_d = nc.dram_tensor("raws", [S, BC, S], dt.float32)

    with tile.TileContext(nc) as tc:
        with tc.tile_pool(name="const", bufs=1) as cpool, \
             tc.tile_pool(name="state", bufs=1) as spool:
            ct = cpool.tile([128, CCOLS], dt.float32)
            nc.sync.dma_start(out=ct[:], in_=consts_d[:])
            def cs(name, w):
                return ct[:, COFS[name]:COFS[name] + w]

            be_t = cs("be", 128)
            bd_t = cs("bd", 128)
            lenb = cs("lenb", 32)
            mka = cs("mka", 256)
            mkb = cs("mkb", 256)
            vlog = cs("vlog", 256)
            indc = cs("indc", 16)
            s2c = cs("s2c", 16)
            lenrow = ct[0:1, COFS["lenrow"]:COFS["lenrow"] + 8]
            ident = cs("ident", 128)
            iotas = ct[0:8, COFS["iotas"]:COFS["iotas"] + 256]
            lenp = ct[0:8, COFS["lenp"]:COFS["lenp"] + 1]

            w2t = cpool.tile([128, 2048], dt.float32)
            vtt = cpool.tile([128, 8], dt.float32)

            uT = cpool.tile([128, 4 * 2048], dt.float32)

            hT = spool.tile([128, 32], dt.float32)
            cT = spool.tile([128, 32], dt.float32)
            xhr = spool.tile([128, 64], dt.float32)
            henc_r = spool.tile([128, 32], dt.float32)
            nc.gpsimd.memset(hT[:], 0.0)
            nc.gpsimd.memset(cT[:], 0.0)
            nc.vector.tensor_scalar(xhr[:], ct[:, 0:64], 0.0, None, OP.mult)
            nc.vector.tensor_scalar(henc_r[:], ct[:, 0:32], 0.0, None, OP.mult)

            rowsp_cm = tc.tile_pool(name="rowsp", bufs=1)
            rowsp = rowsp_cm.__enter__()
            encrows_sb = rowsp.tile([128, BC * 2 * 512], dt.float32)
            with tc.tile_pool(name="eo", bufs=1) as eop:
                encw_cm = tc.tile_pool(name="encw", bufs=1)
                encw = encw_cm.__enter__()
                enc_outT = eop.tile([128, S * 32], dt.float32)  # (s, k, b)
                w1t = eop.tile([128, 2048], dt.float32)
                with tc.tile_pool(name="wsqp", bufs=1) as wsqp:
                    wsq32 = wsqp.tile([128, 2 * 2048 + 8], dt.float32)
                    nc.sync.dma_start(out=wsq32[:], in_=wsq_d[:])
                    nc.vector.tensor_copy(w2t[:], wsq32[:, 0:2048])
                    nc.vector.tensor_copy(w1t[:], wsq32[:, 2048:4096])
                    nc.vector.tensor_copy(vtt[:], wsq32[:, 4096:4104])
                enc_lhsT = encw.tile([128, 6 * G], dt.float32)
                xT = encw.tile([128, 2 * S * BC], dt.float32)  # (ec; s,b)

                with tc.tile_pool(name="wtmp", bufs=2) as wtmp:
                    for kc in range(6):
                        we32 = wtmp.tile([128, G], dt.float32, tag="we32")
                        nc.sync.dma_start(out=we32[:], in_=wenc_d[:, kc * G:(kc + 1) * G])
                        nc.vector.tensor_copy(enc_lhsT[:, kc * G:(kc + 1) * G], we32[:])

                # ---- embedding via one-hot matmul (exact fp32 gather)
                with tc.tile_pool(name="embp", bufs=1) as embp, \
                     tc.tile_pool(name="ohp", bufs=1) as ohp, \
                     tc.tile_pool(name="pse", bufs=1, space="PSUM") as pse:
                    embt = embp.tile([128, 8 * E], dt.float32)
                    nc.sync.dma_start(out=embt[:], in_=embt_d[:])
                    pe0 = pse.tile([128, S * BC], dt.float32, tag="pe0")
                    pe1 = pse.tile([128, S * BC], dt.float32, tag="pe1")
                    pes = [pe0, pe1]
                    for kc in range(8):
                        ohc = ohp.tile([128, S * BC], dt.float32, tag="ohc")
                        nc.sync.dma_start(out=ohc[:],
                                          in_=ohsrc_d[:, kc * S * BC:(kc + 1) * S * BC])
                        for m in range(2):
                            for ns in range(4):
                                nc.tensor.matmul(
                                    pes[m][:, 512 * ns:512 * (ns + 1)],
                                    embt[:, kc * E + 128 * m: kc * E + 128 * (m + 1)],
                                    ohc[:, 512 * ns:512 * (ns + 1)],
                                    start=(kc == 0), stop=(kc == 7))
                    for m in range(2):
                        nc.vector.tensor_copy(
                            xT[:, m * S * BC:(m + 1) * S * BC], pes[m][:])

                # ---- encoder ----
                with tc.tile_pool(name="psg", bufs=2, space="PSUM") as psg, \
                     tc.tile_pool(name="esb", bufs=3) as esb:
                    for t in range(S):
                        pg = psg.tile([128, 128], dt.float32, tag="pg")
                        for kc in [2, 3, 4, 5, 0, 1]:
                            if kc < 2:
                                rhs = xT[:, kc * (S * BC) + t * BC:
                                         kc * (S * BC) + (t + 1) * BC]
                            else:
                                rhs = henc_r[:, 8 * (kc - 2): 8 * (kc - 2) + 8]
                            for m in range(16):
                                nc.tensor.matmul(
                                    pg[:, 8 * m:8 * m + 8],
                                    enc_lhsT[:, kc * G + 128 * m: kc * G + 128 * (m + 1)],
                                    rhs, start=(kc == 2), stop=(kc == 1))
                        pre = esb.tile([128, 128], dt.float32, tag="pre")
                        nc.vector.tensor_tensor(out=pre[:], in0=pg[:], in1=be_t, op=OP.add)
                        sg = esb.tile([128, 96], dt.float32, tag="sg")
                        tg = esb.tile([128, 32], dt.float32, tag="tg")
                        nc.scalar.activation(sg[:], pre[:, 0:96], AF.Sigmoid)
                        nc.scalar.activation(tg[:], pre[:, 96:128], AF.Tanh)
                        t1 = esb.tile([128, 32], dt.float32, tag="t1")
                        c2 = esb.tile([128, 32], dt.float32, tag="c2")
                        nc.vector.tensor_tensor(out=t1[:], in0=sg[:, 0:32], in1=tg[:], op=OP.mult)
                        nc.vector.tensor_tensor(out=c2[:], in0=sg[:, 32:64], in1=cT[:], op=OP.mult)
                        nc.vector.tensor_tensor(out=c2[:], in0=c2[:], in1=t1[:], op=OP.add)
                        tcc = esb.tile([128, 32], dt.float32, tag="tcc")
                        nc.scalar.activation(tcc[:], c2[:], AF.Tanh)
                        h2 = esb.tile([128, 32], dt.float32, tag="h2")
                        nc.vector.tensor_tensor(out=h2[:], in0=sg[:, 64:96], in1=tcc[:], op=OP.mult)
                        msk = esb.tile([128, 32], dt.uint8, tag="msk")
                        mskf32 = esb.tile([128, 32], dt.float32, tag="mskf32")
                        nc.vector.tensor_scalar(msk[:], lenb, float(t), None, OP.is_gt)
                        nc.vector.tensor_scalar(mskf32[:], lenb, float(t), None, OP.is_gt)
                        nc.vector.select(cT[:], msk[:], c2[:], cT[:])
                        nc.vector.select(hT[:], msk[:], h2[:], hT[:])
                        et = enc_outT[:, 32 * t:32 * t + 32]
                        nc.vector.tensor_tensor(out=et, in0=mskf32[:], in1=hT[:], op=OP.mult)
                        nc.vector.tensor_copy(henc_r[:], hT[:])

                encw_cm.__exit__(None, None, None)
                # ---- reformat enc_out to SBUF row form (xnext one-hot source)
                with tc.tile_pool(name="rfps", bufs=2, space="PSUM") as rfps:
                    eview = enc_outT[:].rearrange("p (s k b) -> p k b s", s=S, k=4, b=BC)
                    for b in range(BC):
                        for sh in range(2):
                            for k in range(4):
                                pt = rfps.tile([128, 128], dt.float32, tag="pt")
                                nc.tensor.transpose(
                                    pt[:], eview[:, k, b, 128 * sh:128 * (sh + 1)], ident)
                                nc.vector.tensor_copy(
                                    encrows_sb[:, b * 1024 + sh * 512 + 128 * k:
                                               b * 1024 + sh * 512 + 128 * (k + 1)], pt[:])

                # ---- U = W1 @ enc_out.T ----
                with tc.tile_pool(name="psu", bufs=2, space="PSUM") as psu:
                    erv = enc_outT[:].rearrange("p (s k b) -> p k b s", s=S, k=4, b=BC)
                    for mc in range(4):
                        for nb in range(4):
                            pu = psu.tile([128, 512], dt.float32, tag="pu")
                            for kc in range(4):
                                nc.tensor.matmul(
                                    pu[:],
                                    w1t[:, kc * 512 + 128 * mc: kc * 512 + 128 * (mc + 1)],
                                    erv[:, kc, 2 * nb:2 * nb + 2, :],
                                    start=(kc == 0), stop=(kc == 3))
                            nc.vector.tensor_copy(
                                uT[:, mc * 2048 + nb * 512: mc * 2048 + (nb + 1) * 512],
                                pu[:])

            # ---- decoder ----
            with tc.tile_pool(name="decw", bufs=1) as decw:
                dec_lhsT = decw.tile([128, 8 * G], dt.float32)
                with tc.tile_pool(name="wtmp2", bufs=2) as wtmp2:
                    for kc in range(8):
                        wd32 = wtmp2.tile([128, G], dt.float32, tag="wd32")
                        nc.sync.dma_start(out=wd32[:], in_=wdec_d[:, kc * G:(kc + 1) * G])
                        nc.vector.tensor_copy(dec_lhsT[:, kc * G:(kc + 1) * G], wd32[:])

                with tc.tile_pool(name="psd", bufs=2, space="PSUM") as psd, \
                     tc.tile_pool(name="psq", bufs=1, space="PSUM") as psq, \
                     tc.tile_pool(name="ptxp", bufs=1, space="PSUM") as ptxp, \
                     tc.tile_pool(name="ptrp", bufs=1, space="PSUM") as ptrp, \
                     tc.tile_pool(name="pdt", bufs=2, space="PSUM") as pdtp, \
                     tc.tile_pool(name="dsb", bufs=2) as dsb:
                    nc.vector.tensor_copy(xhr[:, 32:64], hT[:])
                    pg_prev = None
                    for i in range(S):
                        if pg_prev is None:
                            pg = psd.tile([128, 128], dt.float32, tag="pg")
                            for kc in [4, 5, 6, 7]:
                                rhs = xhr[:, 8 * kc: 8 * kc + 8]
                                for m in range(16):
                                    nc.tensor.matmul(
                                        pg[:, 8 * m:8 * m + 8],
                                        dec_lhsT[:, kc * G + 128 * m: kc * G + 128 * (m + 1)],
                                        rhs, start=(kc == 4), stop=False)
                        else:
                            pg = pg_prev
                        for kc in [0, 1, 2, 3]:
                            rhs = xhr[:, 8 * kc: 8 * kc + 8]
                            for m in range(16):
                                nc.tensor.matmul(
                                    pg[:, 8 * m:8 * m + 8],
                                    dec_lhsT[:, kc * G + 128 * m: kc * G + 128 * (m + 1)],
                                    rhs, start=False, stop=(kc == 3))
                        pre = dsb.tile([128, 128], dt.float32, tag="pre")
                        nc.vector.tensor_tensor(out=pre[:], in0=pg[:], in1=bd_t, op=OP.add)
                        sg = dsb.tile([128, 96], dt.float32, tag="sg")
                        tg = dsb.tile([128, 32], dt.float32, tag="tg")
                        nc.scalar.activation(sg[:], pre[:, 0:96], AF.Sigmoid)
                        nc.scalar.activation(tg[:], pre[:, 96:128], AF.Tanh)
                        t1 = dsb.tile([128, 32], dt.float32, tag="t1")
                        nc.vector.tensor_tensor(out=t1[:], in0=sg[:, 0:32], in1=tg[:], op=OP.mult)
                        nc.vector.tensor_tensor(out=cT[:], in0=sg[:, 32:64], in1=cT[:], op=OP.mult)
                        nc.vector.tensor_tensor(out=cT[:], in0=cT[:], in1=t1[:], op=OP.add)
                        tcc = dsb.tile([128, 32], dt.float32, tag="tcc")
                        nc.scalar.activation(tcc[:], cT[:], AF.Tanh)
                        nc.vector.tensor_tensor(out=hT[:], in0=sg[:, 64:96], in1=tcc[:], op=OP.mult)
                        nc.vector.tensor_copy(xhr[:, 32:64], hT[:])

                        pq = psq.tile([128, 32], dt.float32, tag="pq")
                        for kc in range(4):
                            rhs = xhr[:, 32 + 8 * kc: 32 + 8 * kc + 8]
                            for m in range(4):
                                nc.tensor.matmul(
                                    pq[:, 8 * m:8 * m + 8],
                                    w2t[:, kc * 512 + 128 * m: kc * 512 + 128 * (m + 1)],
                                    rhs, start=(kc == 0), stop=(kc == 3))
                        qT = dsb.tile([128, 32], dt.float32, tag="qT")
                        nc.vector.tensor_copy(qT[:], pq[:])

                        pd = pdtp.tile([128, 512], dt.float32, tag="pd")
                        for c in range(4):
                            att = dsb.tile([128, 2048], dt.float32, tag="att")
                            for b in range(BC):
                                nc.vector.tensor_scalar(
                                    att[:, 256 * b:256 * (b + 1)],
                                    uT[:, c * 2048 + 256 * b: c * 2048 + 256 * (b + 1)],
                                    qT[:, 8 * c + b: 8 * c + b + 1], None, OP.add)
                            th = dsb.tile([128, 2048], dt.float32, tag="th")
                            nc.scalar.activation(th[:], att[:], AF.Tanh)
                            for ti in range(2):
                                for j in range(4):
                                    b = 4 * ti + j
                                    nc.tensor.matmul(
                                        pd[32 * j:32 * j + 1, 256 * ti:256 * (ti + 1)],
                                        vtt[:, c: c + 1],
                                        th[:, 256 * b:256 * (b + 1)],
                                        start=(c == 0),
                                        stop=(c == 3),
                                        tile_position=(0, 32 * j))

                        if i + 1 < S:
                            pg_prev = psd.tile([128, 128], dt.float32, tag="pg")
                            for kc in [4, 5, 6, 7]:
                                rhs = xhr[:, 8 * kc: 8 * kc + 8]
                                for m in range(16):
                                    nc.tensor.matmul(
                                        pg_prev[:, 8 * m:8 * m + 8],
                                        dec_lhsT[:, kc * G + 128 * m: kc * G + 128 * (m + 1)],
                                        rhs, start=(kc == 4), stop=False)

                        raw0 = dsb.tile([128, 256], dt.float32, tag="raw0")
                        raw1 = dsb.tile([128, 256], dt.float32, tag="raw1")
                        nc.vector.tensor_copy(raw0[:], pd[:, 0:256])
                        nc.vector.tensor_copy(raw1[:], pd[:, 256:512])
                        nc.sync.dma_start(out=raws_d[i, 0:4, :], in_=raw0[0:128:32, :])
                        nc.sync.dma_start(out=raws_d[i, 4:8, :], in_=raw1[0:128:32, :])

                        sc0 = dsb.tile([128, 256], dt.float32, tag="sc0")
                        sc1 = dsb.tile([128, 256], dt.float32, tag="sc1")
                        nc.vector.tensor_tensor(out=sc0[:], in0=pd[:, 0:256], in1=mka, op=OP.add)
                        nc.vector.tensor_tensor(out=sc1[:], in0=pd[:, 256:512], in1=mkb, op=OP.add)
                        mx0 = dsb.tile([128, 8], dt.float32, tag="mx0")
                        mi0 = dsb.tile([128, 8], dt.uint32, tag="mi0")
                        mx1 = dsb.tile([128, 8], dt.float32, tag="mx1")
                        mi1 = dsb.tile([128, 8], dt.uint32, tag="mi1")
                        nc.vector.max_with_indices(mx0[:], mi0[:], sc0[:])
                        nc.vector.max_with_indices(mx1[:], mi1[:], sc1[:])
                        idxP = dsb.tile([8, 1], dt.uint32, tag="idxP")
                        nc.sync.dma_start(out=idxP[0:4, 0:1], in_=mi0[0:128:32, 0:1])
                        nc.sync.dma_start(out=idxP[4:8, 0:1], in_=mi1[0:128:32, 0:1])
                        indP = dsb.tile([8, 1], dt.float32, tag="indP")
                        nc.vector.tensor_scalar(indP[:], lenp, float(i), None, OP.is_gt)
                        idxPf = dsb.tile([8, 1], dt.float32, tag="idxPf")
                        nc.vector.tensor_copy(idxPf[:], idxP[:])
                        idxPm = dsb.tile([8, 1], dt.float32, tag="idxPm")
                        nc.vector.tensor_tensor(out=idxPm[:], in0=idxPf[:], in1=indP[:],
                                                op=OP.mult)
                        idxu = dsb.tile([8, 1], dt.uint32, tag="idxu")
                        nc.vector.tensor_copy(idxu[:], idxPm[:])
                        nc.sync.dma_start(out=idx_d[0:1, BC * i: BC * i + BC],
                                          in_=idxu[:])
                        ohT = dsb.tile([8, 256], dt.float32, tag="ohT")
                        nc.vector.tensor_scalar(ohT[:], iotas, idxPm[:], None, OP.is_equal)
                        ptr = ptrp.tile([128, 16], dt.float32, tag="ptr")
                        for sh in range(2):
                            nc.tensor.transpose(ptr[:, 8 * sh:8 * sh + 8],
                                                ohT[:, 128 * sh:128 * (sh + 1)],
                                                ident[0:8, 0:8])
                        ohsb = dsb.tile([128, 16], dt.float32, tag="ohsb")
                        nc.vector.tensor_copy(ohsb[:], ptr[:])
                        ptx = ptxp.tile([128, 32], dt.float32, tag="ptx")
                        for b in range(BC):
                            for m in range(4):
                                for sh in range(2):
                                    nc.tensor.matmul(
                                        ptx[:, 8 * m + b: 8 * m + b + 1],
                                        encrows_sb[:, b * 1024 + sh * 512 + 128 * m:
                                                   b * 1024 + sh * 512 + 128 * (m + 1)],
                                        ohsb[:, sh * 8 + b: sh * 8 + b + 1],
                                        start=(sh == 0), stop=(sh == 1))
                        nc.vector.tensor_copy(xhr[:, 0:32], ptx[:])

            rowsp_cm.__exit__(None, None, None)
            # ---- final pass: masked log_softmax ----
            with tc.tile_pool(name="fin", bufs=3) as fsb:
                for ch in range(16):
                    raw = fsb.tile([128, 256], dt.float32, tag="raw")
                    src = raws_d[:].rearrange("(ch q) b s -> ch (q b) s", ch=16, q=16)
                    nc.sync.dma_start(out=raw[:], in_=src[ch])
                    t1f = fsb.tile([128, 256], dt.float32, tag="ft1")
                    nc.vector.tensor_scalar(t1f[:], vlog, indc[:, ch:ch + 1], None, OP.mult)
                    nc.vector.tensor_scalar(t1f[:], t1f[:], s2c[:, ch:ch + 1], None, OP.add)
                    mskf = fsb.tile([128, 256], dt.float32, tag="fmsk")
                    nc.vector.tensor_tensor(out=mskf[:], in0=raw[:], in1=t1f[:], op=OP.add)
                    mx = fsb.tile([128, 1], dt.float32, tag="fmx")
                    nc.vector.tensor_reduce(out=mx[:], in_=mskf[:],
                                            axis=mybir.AxisListType.X, op=OP.max)
                    nmx = fsb.tile([128, 1], dt.float32, tag="fnmx")
                    nc.vector.tensor_scalar(nmx[:], mx[:], -1.0, None, OP.mult)
                    ex = fsb.tile([128, 256], dt.float32, tag="fex")
                    se = fsb.tile([128, 1], dt.float32, tag="fse")
                    nc.scalar.activation(ex[:], mskf[:], AF.Exp, bias=nmx[:], scale=1.0,
                                         accum_out=se[:])
                    ls = fsb.tile([128, 1], dt.float32, tag="fls")
                    nc.scalar.activation(ls[:], se[:], AF.Ln)
                    tot = fsb.tile([128, 1], dt.float32, tag="ftot")
                    nc.vector.tensor_tensor(out=tot[:], in0=mx[:], in1=ls[:], op=OP.add)
                    outt = fsb.tile([128, 256], dt.float32, tag="fout")
                    nc.vector.tensor_scalar(outt[:], mskf[:], tot[:], None, OP.subtract)
                    dstv = scores_d[:].rearrange("(ch q) b s -> ch (q b) s", ch=16, q=16)
                    nc.sync.dma_start(out=dstv[ch], in_=outt[:])

    nc.finalize()
    return nc


def _prep(inputs):
    src = np.asarray(inputs["src"], np.int32)
    src_len = np.asarray(inputs["src_len"], np.int32)
    emb = np.asarray(inputs["emb"], np.float32)
    W_ih_e = np.asarray(inputs["W_ih_e"], np.float32)
    W_hh_e = np.asarray(inputs["W_hh_e"], np.float32)
    b_e = np.asarray(inputs["b_e"], np.float32)
    W_ih_d = np.asarray(inputs["W_ih_d"], np.float32)
    W_hh_d = np.asarray(inputs["W_hh_d"], np.float32)
    b_d = np.asarray(inputs["b_d"], np.float32)
    W1 = np.asarray(inputs["W1"], np.float32)
    W2 = np.asarray(inputs["W2"], np.float32)
    vt = np.asarray(inputs["vt"], np.float32)

    perm = np.concatenate([np.arange(0, 512), np.arange(512, 1024),
                           np.arange(1536, 2048), np.arange(1024, 1536)])

    def lhsT_tiles(Wf, nk):
        t = Wf.T.reshape(nk, 128, G)
        return np.ascontiguousarray(t.transpose(1, 0, 2).reshape(128, nk * G))

    wenc = lhsT_tiles(np.concatenate([W_ih_e[perm], W_hh_e[perm]], axis=1), 6)
    wdec = lhsT_tiles(np.concatenate([W_ih_d[perm], W_hh_d[perm]], axis=1), 8)

    emb_pad = np.zeros((1024, E), np.float32)
    emb_pad[0:1001] = emb
    embT = np.zeros((128, 8 * E), np.float32)
    for kc in range(8):
        embT[:, kc * E:(kc + 1) * E] = emb_pad[128 * kc:128 * (kc + 1), :]

    def sq_tiles(Wm):
        t = Wm.T.reshape(4, 128, 512)
        return np.ascontiguousarray(t.transpose(1, 0, 2).reshape(128, 4 * 512))

    vt_pack = np.zeros((128, 8), np.float32)
    for kc in range(4):
        vt_pack[:, kc] = vt[128 * kc:128 * (kc + 1)]
    wsq = np.concatenate([sq_tiles(W2), sq_tiles(W1), vt_pack], axis=1)

    bep = b_e[perm]
    bdp = b_d[perm]
    be_tile = np.zeros((128, 128), np.float32)
    bd_tile = np.zeros((128, 128), np.float32)
    for m in range(16):
        be_tile[:, 8 * m:8 * m + 8] = bep[128 * m:128 * (m + 1)][:, None]
        bd_tile[:, 8 * m:8 * m + 8] = bdp[128 * m:128 * (m + 1)][:, None]

    in_maps = []
    auxs = []
    for core in range(NCORES):
        sl = src_len[core * BC:(core + 1) * BC]
        sv = src[core * BC:(core + 1) * BC]

        lenb = np.zeros((128, 32), np.float32)
        for k in range(4):
            lenb[:, 8 * k:8 * k + 8] = sl[None, :].astype(np.float32)

        valid = (np.arange(S)[None, :] < sl[:, None])  # [BC, S]
        mka = np.zeros((128, 256), np.float32)
        mkb = np.zeros((128, 256), np.float32)
        for j in range(4):
            mka[32 * j] = np.where(valid[j], 0.0, NEG_BIG)
            mkb[32 * j] = np.where(valid[4 + j], 0.0, NEG_BIG)

        vlogv = np.zeros((128, 256), np.float32)
        for p in range(128):
            vlogv[p] = np.where(valid[p % 8], 0.0, LOG_EPS)
        indcv = np.zeros((128, 16), np.float32)
        s2cv = np.zeros((128, 16), np.float32)
        for ch in range(16):
            for p in range(128):
                row = ch * 128 + p
                ind = 1.0 if (row // 8) < sl[row % 8] else 0.0
                indcv[p, ch] = ind
                s2cv[p, ch] = LOG_EPS * (1.0 - ind)

        lenrowv = np.zeros((128, 8), np.float32)
        lenrowv[0] = sl.astype(np.float32)
        identv = np.eye(128, dtype=np.float32)
        iotasv = np.zeros((128, 256), np.float32)
        iotasv[0:8] = np.arange(256, dtype=np.float32)[None, :]
        lenpv = np.zeros((128, 1), np.float32)
        lenpv[0:8, 0] = sl.astype(np.float32)
        consts = np.concatenate([be_tile, bd_tile, lenb, mka, mkb, vlogv,
                                 indcv, s2cv, lenrowv, identv, iotasv, lenpv], axis=1)

        # one-hot of src tokens: [128, 8*2048], tile kc col sb -> (row==128*kc+p)
        sbord = sv.T.reshape(-1)  # (s,b) -> token
        ohsrc = np.zeros((128, 8 * S * BC), np.float32)
        rows = sbord.astype(np.int64)
        for kc in range(8):
            inblk = (rows >= 128 * kc) & (rows < 128 * (kc + 1))
            cols = np.nonzero(inblk)[0]
            ohsrc[rows[cols] - 128 * kc, kc * S * BC + cols] = 1.0

        in_maps.append({"wenc": wenc, "wdec": wdec, "wsq": wsq.astype(np.float32),
                        "consts": consts.astype(np.float32),
                        "embT": embT, "ohsrc": ohsrc})
        auxs.append({"valid": valid})
    return in_maps, auxs


def kernel(**inputs):
    import concourse.bass_utils as bass_utils

    if "nc" not in _CACHE:
        _CACHE["nc"] = _build()
    nc = _CACHE["nc"]

    in_maps, auxs = _prep(inputs)
    res = bass_utils.run_bass_kernel_spmd(nc, in_maps, list(range(NCORES)))

    scores = np.zeros((B, S, S), np.float32)
    idxs = np.zeros((B, S), np.int32)
    valid_out = np.zeros((B, S), np.int32)
    for core in range(NCORES):
        out = res.results[core]
        scores[core * BC:(core + 1) * BC] = out["scores"].transpose(1, 0, 2)
        idxs[core * BC:(core + 1) * BC] = out["idx"].reshape(S, BC).T.astype(np.int32)
        valid_out[core * BC:(core + 1) * BC] = auxs[core]["valid"].astype(np.int32)
    return scores, idxs, valid_out


if __name__ == "__main__":
    data = dict(np.load(os.path.join(os.path.dirname(__file__), "inputs.npz")))
    out = kernel(**data)
    print([o.shape for o in out])
